# revision 1
# baseline (speedup 1.0000x reference)
"""Trainium2 Bass kernel for nn_Critic_Mix (dense MLP critic with teacher mixing).

Strategy:
  - Pure data parallel: batch (B=262144) sharded across 8 cores (32768 rows each).
  - Host-side prep:
      * xu = concat(x, u) pre-transposed to feature-major [128, B] so the matmul
        moving operand DMAs straight into SBUF with no on-chip transpose.
      * Layer-1 mixsum is linear in xu -> folded into one effective main weight.
      * Teacher layer-2 weights pre-scaled by c_k = m*P_k; relu commutes with the
        positive scale, and layer-3 teacher weights absorb 1/c_k exactly, so the
        main-path mixsums become plain PSUM accumulations of matmuls.
      * Layer-3 teacher outputs feed only the mixsum -> accumulate all 5 matmuls
        (main + 4 teachers) into a single PSUM bank per head.
      * Layer-4 ([64]->[1]) uses 16 one-hot-column weight variants so 8 row-tiles
        x 2 heads accumulate into distinct partitions of one PSUM bank; one
        evacuation + one DMA per 8 tiles. Final scalar biases added on host.
  - On-chip per 512-row tile: 40 matmuls (PE), 22 bias+relu PSUM->SBUF
    evacuations split between ScalarE (activation, free bias) and VectorE
    (tensor_scalar add+max, free bias).
"""

import os
import sys
from contextlib import ExitStack

import numpy as np

for _p in ("/opt/trn_rl_repo",):
    if _p not in sys.path and os.path.isdir(_p):
        sys.path.insert(0, _p)

import concourse.bass as bass
import concourse.tile as tile
from concourse import bacc, mybir
from concourse._compat import with_exitstack
from concourse.bass import ts
from concourse.bass_utils import run_bass_kernel_spmd

# Problem constants (hardcoded; kernel.py must be self-contained).
B = 262144
NCORES = 8
CB = B // NCORES          # rows per core
NT = 512                  # moving-dim tile (one PSUM bank of fp32)
DIN = 128                 # xu feature dim (96 + 32)
H1 = 128
H2 = 64
K = 4

F32 = mybir.dt.float32
AF = mybir.ActivationFunctionType
ALU = mybir.AluOpType


# ---------------------------------------------------------------------------
# Weight / bias column layouts (shared by host packing and kernel body)
# ---------------------------------------------------------------------------
def _wlayout():
    off = {}
    cur = 0
    for h in (0, 1):
        for k in range(K):
            off[f"l1t{h}{k}"] = (cur, 128)
            cur += 128
        off[f"l1m{h}"] = (cur, 128)
        cur += 128
        for k in range(K):
            off[f"l2t{h}{k}"] = (cur, 128)
            cur += 128
        off[f"l2m{h}"] = (cur, 128)
        cur += 128
        for k in range(K):
            off[f"l3t{h}{k}"] = (cur, 64)
            cur += 64
        off[f"l3m{h}"] = (cur, 64)
        cur += 64
    off["l4"] = (cur, 16 * 16)  # 16 variants x 16 cols
    cur += 16 * 16
    return off, cur


def _blayout():
    off = {}
    cur = 0
    for h in (0, 1):
        for k in range(K):
            off[f"b1t{h}{k}"] = cur
            cur += 1
        off[f"b1m{h}"] = cur
        cur += 1
        for k in range(K):
            off[f"b2t{h}{k}"] = cur
            cur += 1
        off[f"b2m{h}"] = cur
        cur += 1
        off[f"b3{h}"] = cur
        cur += 1
    return off, cur


WOFF, WCOLS = _wlayout()
BOFF, BCOLS = _blayout()


# ---------------------------------------------------------------------------
# Host-side parameter folding
# ---------------------------------------------------------------------------
def prepare_params(inputs):
    """Pack folded weights/biases. Returns (wts [128,WCOLS], biasv [128,BCOLS],
    (b4, b8)) all float32."""
    m = np.float32(np.asarray(inputs["mix_factor"]).reshape(-1)[0])
    P = np.asarray(inputs["teacher_P"], np.float32).reshape(K)
    om = np.float32(1.0) - m
    c = m * P  # [K], >= 0

    wts = np.zeros((128, WCOLS), np.float32)
    biasv = np.zeros((128, BCOLS), np.float32)

    def wput(name, arr):
        o, w = WOFF[name]
        arr = np.asarray(arr, np.float32)
        assert arr.shape[1] == w, (name, arr.shape, w)
        wts[: arr.shape[0], o : o + w] = arr

    def bput(name, vec):
        vec = np.asarray(vec, np.float32).reshape(-1)
        biasv[: vec.shape[0], BOFF[name]] = vec

    heads = [
        ("W1", "b1", "W2", "b2", "W3", "b3", "W4", "b4", "tW1", "tb1", "tW2", "tb2", "tW3", "tb3"),
        ("W5", "b5", "W6", "b6", "W7", "b7", "W8", "b8", "tW5", "tb5", "tW6", "tb6", "tW7", "tb7"),
    ]
    out_biases = []
    l4 = np.zeros((64, 16 * 16), np.float32)
    for h, names in enumerate(heads):
        (Wa, ba, Wb, bb, Wc, bc, Wd, bd, tWa, tba, tWb, tbb, tWc, tbc) = (
            np.asarray(inputs[n], np.float32) for n in names
        )
        # L1: teachers raw; main folded with the (linear) layer-1 mixsum.
        for k in range(K):
            wput(f"l1t{h}{k}", tWa[k].T)
            bput(f"b1t{h}{k}", tba[k])
        W1eff = om * Wa + m * np.einsum("k,koi->oi", P, tWa)
        b1eff = om * ba + m * (P[:, None] * tba).sum(0)
        wput(f"l1m{h}", W1eff.T)
        bput(f"b1m{h}", b1eff)
        # L2: teachers pre-scaled by c_k (relu commutes; L3 absorbs 1/c_k).
        for k in range(K):
            wput(f"l2t{h}{k}", (c[k] * tWb[k]).T)
            bput(f"b2t{h}{k}", c[k] * tbb[k])
        wput(f"l2m{h}", (om * Wb).T)
        bput(f"b2m{h}", om * bb + (c[:, None] * tbb).sum(0))
        # L3: single accumulated bank; teacher weights unscaled (1/c_k * m*P_k = 1).
        for k in range(K):
            wput(f"l3t{h}{k}", tWc[k].T)
        wput(f"l3m{h}", (om * Wc).T)
        bput(f"b3{h}", om * bc + m * (P[:, None] * tbc).sum(0))
        # L4 one-hot-column variants: variant v = 8*h + j has w4 at col v.
        for j in range(8):
            v = 8 * h + j
            l4[:, v * 16 + v] = Wd[0]
        out_biases.append(np.float32(bd[0]))

    o, w = WOFF["l4"]
    wts[:64, o : o + w] = l4
    return wts, biasv, out_biases


def prepare_xut(inputs):
    x = np.asarray(inputs["x"], np.float32)
    u = np.asarray(inputs["u"], np.float32)
    xu = np.concatenate([x, u], axis=1)  # [B, 128]
    return np.ascontiguousarray(xu.T)  # [128, B]


# ---------------------------------------------------------------------------
# Kernel body
# ---------------------------------------------------------------------------
@with_exitstack
def _critic_body(ctx: ExitStack, tc, out_ap, xu_ap, wts_ap, bias_ap, tiles: int):
    nc = tc.nc

    const = ctx.enter_context(tc.tile_pool(name="const", bufs=1))
    xup = ctx.enter_context(tc.tile_pool(name="xup", bufs=3))
    actp = ctx.enter_context(tc.tile_pool(name="actp", bufs=2))
    psp = ctx.enter_context(tc.tile_pool(name="psp", bufs=6, space=bass.MemorySpace.PSUM))
    ps4p = ctx.enter_context(tc.tile_pool(name="ps4p", bufs=2, space=bass.MemorySpace.PSUM))

    wts = const.tile([128, WCOLS], F32)
    nc.gpsimd.dma_start(wts[:], wts_ap[:])
    biasv = const.tile([128, BCOLS], F32)
    nc.gpsimd.dma_start(biasv[:], bias_ap[:])

    def w(name):
        o, wd = WOFF[name]
        return wts[:, o : o + wd]

    def bvec(name, parts=128):
        col = BOFF[name]
        return biasv[0:parts, col : col + 1]

    def evac(dst, src, bname, eng, parts=128):
        # dst = relu(src + bias)
        if eng == "act":
            nc.scalar.activation(dst, src, AF.Relu, bias=bvec(bname, parts), scale=1.0)
        else:
            nc.vector.tensor_scalar(
                out=dst, in0=src, scalar1=bvec(bname, parts), scalar2=0.0,
                op0=ALU.add, op1=ALU.max,
            )

    l4o, _ = WOFF["l4"]
    ps4 = None
    for t in range(tiles):
        xu = xup.tile([128, NT], F32, tag="xu")
        nc.gpsimd.dma_start(xu[:], xu_ap[:, ts(t, NT)])
        j = t % 8
        if j == 0:
            ps4 = ps4p.tile([16, NT], F32, tag="ps4")
        for h in (0, 1):
            # ---- L1: 4 teachers + folded main
            rh1 = []
            for k in range(K):
                ps = psp.tile([128, NT], F32, tag="ps")
                nc.tensor.matmul(ps[:], w(f"l1t{h}{k}"), xu[:], start=True, stop=True)
                r = actp.tile([128, NT], F32, tag=f"rh1_{h}_{k}")
                evac(r[:], ps[:], f"b1t{h}{k}", "act" if k % 2 == 0 else "dve")
                rh1.append(r)
            psm = psp.tile([128, NT], F32, tag="ps")
            nc.tensor.matmul(psm[:], w(f"l1m{h}"), xu[:], start=True, stop=True)
            h1 = actp.tile([128, NT], F32, tag=f"h1_{h}")
            evac(h1[:], psm[:], f"b1m{h}", "act")

            # ---- L2: teachers (scaled) + main accumulated in PSUM
            ps2m = psp.tile([128, NT], F32, tag="ps")
            nc.tensor.matmul(ps2m[:], w(f"l2m{h}"), h1[:], start=True, stop=False)
            rh2 = []
            for k in range(K):
                ps2k = psp.tile([128, NT], F32, tag="ps")
                nc.tensor.matmul(ps2k[:], w(f"l2t{h}{k}"), rh1[k][:], start=True, stop=True)
                nc.tensor.matmul(ps2m[:], w(f"l2t{h}{k}"), rh1[k][:], start=False, stop=(k == K - 1))
                r2 = actp.tile([128, NT], F32, tag=f"rh2_{h}_{k}")
                evac(r2[:], ps2k[:], f"b2t{h}{k}", "act" if k % 2 == 0 else "dve")
                rh2.append(r2)
            h2 = actp.tile([128, NT], F32, tag=f"h2_{h}")
            evac(h2[:], ps2m[:], f"b2m{h}", "dve")

            # ---- L3: main + 4 teachers accumulated into one [64, NT] bank
            ps3 = psp.tile([64, NT], F32, tag="ps")
            nc.tensor.matmul(ps3[:], w(f"l3m{h}"), h2[:], start=True, stop=False)
            for k in range(K):
                nc.tensor.matmul(ps3[:], w(f"l3t{h}{k}"), rh2[k][:], start=False, stop=(k == K - 1))
            h3 = actp.tile([64, NT], F32, tag=f"h3_{h}")
            evac(h3[:], ps3[:], f"b3{h}", "act", parts=64)

            # ---- L4: accumulate tile scalar-outputs into partition 8h+j of ps4
            v = 8 * h + j
            nc.tensor.matmul(
                ps4[:], wts[0:64, l4o + v * 16 : l4o + (v + 1) * 16], h3[:],
                start=(j == 0 and h == 0), stop=(j == 7 and h == 1),
            )
        if j == 7:
            o = actp.tile([16, NT], F32, tag="osb")
            nc.scalar.activation(o[:], ps4[:], AF.Copy, bias=0.0, scale=1.0)
            nc.gpsimd.dma_start(out_ap[:, ts(t // 8, NT)], o[:])


def build_nc(cb=CB):
    """Build + compile the per-core program for cb rows (cb % (8*NT) == 0)."""
    assert cb % (8 * NT) == 0
    tiles = cb // NT
    nc = bacc.Bacc(
        "TRN2",
        target_bir_lowering=False,
        debug=False,
        enable_asserts=False,
        num_devices=NCORES,
    )
    xu_ap = nc.dram_tensor("xut", [128, cb], F32, kind="ExternalInput").ap()
    wts_ap = nc.dram_tensor("wts", [128, WCOLS], F32, kind="ExternalInput").ap()
    bias_ap = nc.dram_tensor("biasv", [128, BCOLS], F32, kind="ExternalInput").ap()
    out_ap = nc.dram_tensor("out", [16, cb // 8], F32, kind="ExternalOutput").ap()
    with tile.TileContext(nc) as tc:
        _critic_body(tc, out_ap, xu_ap, wts_ap, bias_ap, tiles)
    nc.compile()
    return nc


def unscramble_out(out_c):
    """[16, cb//8] device layout -> (y1 [cb], y2 [cb])."""
    g = out_c.shape[1] // NT
    ys = []
    for h in (0, 1):
        a = out_c[8 * h : 8 * h + 8].reshape(8, g, NT)
        ys.append(np.ascontiguousarray(a.transpose(1, 0, 2)).reshape(-1))
    return ys


_NC_CACHE = {}
LAST_RESULTS = None  # BassKernelResults of the most recent run (for profiling)


def kernel(**inputs):
    global LAST_RESULTS
    wts, biasv, (b4, b8) = prepare_params(inputs)
    xut = prepare_xut(inputs)

    if CB not in _NC_CACHE:
        _NC_CACHE[CB] = build_nc(CB)
    nc = _NC_CACHE[CB]

    in_maps = [
        {
            "xut": np.ascontiguousarray(xut[:, c * CB : (c + 1) * CB]),
            "wts": wts,
            "biasv": biasv,
        }
        for c in range(NCORES)
    ]
    res = run_bass_kernel_spmd(
        nc,
        in_maps,
        list(range(NCORES)),
        trace=bool(os.environ.get("BASS_TRACE")),
    )
    LAST_RESULTS = res

    y1 = np.empty(B, np.float32)
    y2 = np.empty(B, np.float32)
    for c in range(NCORES):
        a, b = unscramble_out(res.results[c]["out"])
        y1[c * CB : (c + 1) * CB] = a
        y2[c * CB : (c + 1) * CB] = b
    y1 += b4
    y2 += b8
    return (y1[:, None], y2[:, None])
